# revision 39
# baseline (speedup 1.0000x reference)
"""GPT-2 block (B=2, T=2048, C=768, H=12) on 8 Trainium2 NeuronCores.

Sharding: data-parallel over batch (2) x 4-way query-tile split per batch.
Each core computes K/V for its full batch (no on-chip collectives) and runs
attention + MLP for 4 of the 16 query tiles, interleaved {g, 7-g, 8+g, 15-g}
so the causal-attention work is identical across cores.  The SPMD program is
uniform: per-core differences are pushed into the data via a k-tile
permutation of the sequence plus per-core causal masks.

Precision/layout highlights (v2):
- Every GEMM is fp8 e4m3 DoubleRow (2 k-rows/cycle): QKV, proj, fc, fc2 and
  the attention A*V.  Weights are pre-scaled by 32 on the host; LayerNorm
  gains/biases are folded into the adjacent weights/biases host-side, the
  K-bias is dropped (softmax-invariant) and the V-bias rides b_proj.
- Attention: scores are computed transposed (S^T [k,q]) in kp-PAIRS so the
  exp() output is directly the fp8 DoubleRow moving operand of the fused
  A*V matmul whose stationary is [V | 64 ones-columns] - the softmax
  denominator lands in psum rows 64:128, giving one full-partition
  reciprocal + one multiply per head pair.
- Causal masks are applied on the PE: a matmul with stationary -2^20*I and
  per-core (1-mask) moving data accumulates -2^20 into masked score slots
  before exp (data-driven masking, uniform SPMD program).
- The attention residual carries a 32x scale (x_own pre-scaled on host);
  LayerNorm is scale-invariant, so the scale rides through to the output,
  which the host divides by 32.
"""

import sys

sys.path.insert(0, "/opt/trn_rl_repo")

import numpy as np
import ml_dtypes

import bass_rust
import concourse.bass as bass
import concourse.bacc as bacc
import concourse.tile as tile
from concourse import mybir
from concourse.vector_clock import ScopedClock

BF16 = ml_dtypes.bfloat16
F32 = mybir.dt.float32
BF = mybir.dt.bfloat16
F8 = mybir.dt.float8e4
NP_F8 = mybir.dt.np(F8)

B, T, C, H = 2, 2048, 768, 12
HD = C // H  # 64
DFF = 4 * C  # 3072
TT = T // 128  # 16 token tiles
CT = C // 128  # 6 feature tiles
KT = C // 256  # 3 DoubleRow k-tiles over C
KT2 = DFF // 256  # 12 DoubleRow k-tiles over DFF
FT = DFF // 128  # 24
KPP = TT // 2  # 8 key-tile pairs
QPOS = (3, 7, 11, 15)  # fixed positions of this core's query tiles
NQ = 512  # queries per core
WS = 32.0  # fp8 weight pre-scale
MBIG = float(2**20)  # mask magnitude (scaled to -128 at exp input)
AF = mybir.ActivationFunctionType
ALU = mybir.AluOpType
DR = mybir.MatmulPerfMode.DoubleRow

# ---------------------------------------------------------------------------
# Tile exit-drain fix: the final SP drain carries one wait per live logical
# processor, but TRN2 ISA instructions hold at most 1 embedded sync wait in
# this toolchain. Split the waits across a chain of SP drains.
# ---------------------------------------------------------------------------
_MAX_WAITS = 1


def _drain_and_barrier(self, tick_clock, wait_clock):
    drain_inst = self.nc.sync.drain()
    wait_clock.add_sem_waits(
        drain_inst.ins, ScopedClock({None: tick_clock.global_clock})
    )
    si = drain_inst.ins.sync_info
    if si is not None and len(si.on_wait) > _MAX_WAITS:
        waits = list(si.on_wait)
        drain_inst.ins.sync_info = bass_rust.SyncInfo(
            on_wait=waits[:_MAX_WAITS], on_update=list(si.on_update)
        )
        rest = waits[_MAX_WAITS:]
        for i in range(0, len(rest), _MAX_WAITS):
            extra = self.nc.sync.drain()
            extra.ins.sync_info = bass_rust.SyncInfo(
                on_wait=rest[i : i + _MAX_WAITS], on_update=[]
            )
    self.nc.all_engine_barrier()
    assert self.sems is not None
    popped = self.nc._tile_sem_poison_stack.pop()
    assert popped is self._sem_poison
    self.nc.clear_and_free_semaphores(list(self.sems.allocated().values()))
    self.nc.all_engine_barrier()


tile.TileContext._drain_and_barrier = _drain_and_barrier


# ---------------------------------------------------------------------------
# Per-core sharding layout (host side)
# ---------------------------------------------------------------------------
def core_layout(g):
    """For group index g (0..3): (qtiles sorted, perm) with the core's query
    tiles at positions QPOS and every tile's causal prefix placed before it."""
    qtiles = sorted([g, 7 - g, 8 + g, 15 - g])
    posmap = dict(zip(QPOS, qtiles))
    rest = iter([t for t in range(TT) if t not in qtiles])
    perm = [posmap[p] if p in posmap else next(rest) for p in range(TT)]
    for j, a in enumerate(qtiles):
        assert set(range(a + 1)) <= set(perm[: QPOS[j] + 1]), (g, j, perm)
    return qtiles, perm


def core_maskneg(qtiles, perm):
    """maskrep[j, :, hi, r, :] = causal mask for k-position kp=2j+r against
    query tile bi=kp//4 (first in-suffix 128-query block), replicated over
    the head (hi) axis. Multiplied into the exp output on the DVE."""
    mrep = np.zeros((KPP, 128, 2, 2, 128), dtype=BF16)
    for kp in range(TT):
        tk = perm[kp] * 128 + np.arange(128)[:, None]
        a = qtiles[kp // 4]
        tq = a * 128 + np.arange(128)[None, :]
        m = (tk <= tq).astype(np.float32)
        mrep[kp // 2, :, 0, kp % 2, :] = m
        mrep[kp // 2, :, 1, kp % 2, :] = m
    return mrep


def pack_dr(W):
    """[K, N] fp32 -> DoubleRow-paired fp8 [K/256, 128, 2, N], pre-scaled.
    Logical k = 256*kt + 128*r + p."""
    K, N = W.shape
    Wp = (np.asarray(W, np.float32) * WS).reshape(K // 256, 2, 128, N)
    return np.ascontiguousarray(Wp.transpose(0, 2, 1, 3)).astype(NP_F8)


def _chunks(cs):
    """Query-column chunks (<=256 wide) covering [cs, NQ)."""
    if NQ - cs > 256:
        return [(cs, cs + 256), (cs + 256, NQ)]
    return [(cs, NQ)]


# ---------------------------------------------------------------------------
# The Bass program (identical for all 8 cores)
# ---------------------------------------------------------------------------
def build_program():
    nc = bacc.Bacc("TRN2")

    d_x = nc.dram_tensor("x_perm", [T, C], BF, kind="ExternalInput")
    d_xo = nc.dram_tensor("x_own32", [NQ, C], BF, kind="ExternalInput")
    d_mneg = nc.dram_tensor("maskrep", [KPP, 128, 2, 2, 128], BF, kind="ExternalInput")
    d_wq = nc.dram_tensor("wq", [KT, 128, 2, C], F8, kind="ExternalInput")
    d_wk = nc.dram_tensor("wk", [KT, 128, 2, C], F8, kind="ExternalInput")
    d_wv = nc.dram_tensor("wv", [KT, 128, 2, C], F8, kind="ExternalInput")
    d_wp = nc.dram_tensor("wp", [KT, 128, 2, C], F8, kind="ExternalInput")
    d_wfc = nc.dram_tensor("wfc", [C, DFF], BF, kind="ExternalInput")
    d_wfc2 = nc.dram_tensor("wfc2", [DFF, C], BF, kind="ExternalInput")
    # [ident | -2^20*ident]
    d_idents = nc.dram_tensor("idents", [128, 256], BF, kind="ExternalInput")
    # [bq32 (CT) | bfcc (FT) | bfc2b*32 (C)]
    d_cn = nc.dram_tensor("consts", [128, CT + FT + C], F32, kind="ExternalInput")
    d_out = nc.dram_tensor("out", [NQ, C], F32, kind="ExternalOutput")

    with tile.TileContext(nc) as tc:
        _body(nc, tc, locals())
    nc.compile()
    return nc


def _body(nc, tc, d):
    def pool(name, **kw):
        return tc.tile_pool(name=name, **kw)

    with (
        pool("const", bufs=1) as constp,
        pool("pers", bufs=1) as pers,
        pool("small", bufs=6) as small,
    ):
        # ---- constants ---------------------------------------------------
        idents = constp.tile([128, 2, 128], BF)
        nc.scalar.dma_start(
            idents[:], d["d_idents"][:].rearrange("p (a b) -> p a b", b=128)
        )
        ident, identn = idents[:, 0, :], idents[:, 1, :]
        eps = constp.tile([128, 1], F32)
        nc.vector.memset(eps[:], 1e-5)
        cn = constp.tile([128, CT + FT + C], F32)
        nc.scalar.dma_start(cn[:], d["d_cn"][:])
        bq32 = cn[:, 0:CT]
        bfcc = cn[:, CT : CT + FT]
        bfc2b = cn[:, CT + FT :]
        mrep = constp.tile([128, KPP, 2, 2, 128], BF)
        nc.gpsimd.dma_start(mrep[:], d["d_mneg"][:].rearrange("j p a r q -> p j a r q"))

        # ---- persistent tiles --------------------------------------------
        wp8 = [pers.tile([128, 2, C], F8, tag=f"wp{t}", name=f"wp{t}") for t in range(KT)]
        wfc_t = pers.tile([128, CT, DFF], BF, tag="wfct", name="wfct")
        wfc = [wfc_t[:, t, :] for t in range(CT)]
        xo32 = pers.tile([128, 4, C], BF, tag="xo", name="xo")
        kT = [pers.tile([128, T], BF, tag=f"kT{t}", name=f"kT{t}") for t in range(CT)]
        qT = [pers.tile([128, NQ], BF, tag=f"qT{t}", name=f"qT{t}") for t in range(CT)]
        xnT8 = [pers.tile([128, 2, T], F8, tag=f"xnT{t}", name=f"xnT{t}") for t in range(KT)]
        xnTq8 = [pers.tile([128, 2, NQ], F8, tag=f"xnTq{t}", name=f"xnTq{t}") for t in range(KT)]
        yT = [pers.tile([128, 2, NQ], F8, tag=f"yT{t}", name=f"yT{t}") for t in range(KT)]
        mvall = pers.tile([128, TT, 2], F32, tag="mvall", name="mvall")
        x2 = pers.tile([128, 4, C], F32, tag="x2", name="x2")
        rstd1 = pers.tile([128, TT], F32, tag="rstd1", name="rstd1")

        # ---- input DMAs (issued up-front; per-queue FIFO sets priority) --
        for t in range(KT):
            nc.scalar.dma_start(wp8[t][:], d["d_wp"][t])
        nc.gpsimd.dma_start(
            xo32[:], d["d_xo"][:].rearrange("(t p) c -> p t c", p=128)
        )

        # =============== phase 1: LN1 + transpose, feature-major fp8 ======
        wq8 = [pers.tile([128, 2, C], F8, tag=f"wq{t}", name=f"wq{t}") for t in range(KT)]
        PH1_MARKER = 1
        wk8 = [pers.tile([128, 2, C], F8, tag=f"wk{t}", name=f"wk{t}") for t in range(KT)]
        wv8 = [pers.tile([128, 2, C], F8, tag=f"wv{t}", name=f"wv{t}") for t in range(KT)]
        for t in range(KT):
            nc.scalar.dma_start(wk8[t][:], d["d_wk"][t])
        for t in range(KT):
            nc.scalar.dma_start(wv8[t][:], d["d_wv"][t])
        for t in range(KT):
            nc.scalar.dma_start(wq8[t][:], d["d_wq"][t])
        attn2_cm = tc.tile_pool(name="attn2", bufs=1)
        attn2 = attn2_cm.__enter__()
        # V2[j]: DoubleRow stationary over key pairs (tiles 2j, 2j+1):
        # per head 128 cols = [64 V | 64 ones*WS]
        V2 = [
            attn2.tile([128, 2, H * 128], F8, tag=f"V2{j}", name=f"V2{j}")
            for j in range(KPP)
        ]
        for j in range(KPP):
            v4 = V2[j][:].rearrange("p r (h e) -> p r h e", e=128)
            nc.gpsimd.memset(v4[:, :, :, HD:], WS)
        with (
            pool("ph1x", bufs=3) as ph1x,
            pool("ph1s", bufs=2) as ph1s,
        ):
            nc.gpsimd.dma_start(
                wfc_t[:], d["d_wfc"][:].rearrange("(t p) c -> p t c", p=128)
            )


            with (
                pool("ph1t", bufs=1, space="PSUM") as ph1t,
                pool("ph2k", bufs=2, space="PSUM") as ph2k,
            ):
                # PE warm-up: keep the HAM activity monitor at full clock
                warm = ph2k.tile([128, 512], F32, tag="pqk", name="warm")
                for _ in range(128):
                    nc.tensor.matmul(warm[:, 0:128], ident, ident)

                for ttg in range(4):
                    xt = ph1x.tile([128, 4, C], BF, tag="xt", name="xt")
                    nc.sync.dma_start(
                        xt[:],
                        d["d_x"][ttg * 512 : (ttg + 1) * 512, :].rearrange(
                            "(t p) c -> p t c", p=128
                        ),
                    )
                    # LN stats for the 4 tiles of this group
                    for ti in range(4):
                        tt = ttg * 4 + ti
                        stats = ph1s.tile([128, 2, 6], F32, tag="bnst", name="bnst")
                        xg = xt[:, ti, :].rearrange("p (a b) -> p a b", b=384)
                        for a in range(2):
                            nc.vector.bn_stats(out=stats[:, a, :], in_=xg[:, a, :])
                        nc.vector.bn_aggr(out=mvall[:, tt, :], in_=stats[:])
                    sd = ph1s.tile([128, 4], F32, tag="sd", name="sd")
                    nc.scalar.activation(
                        out=sd[:],
                        in_=mvall[:, 4 * ttg : 4 * ttg + 4, 1],
                        func=AF.Sqrt,
                        bias=eps[:],
                    )
                    nc.vector.reciprocal(
                        out=rstd1[:, 4 * ttg : 4 * ttg + 4], in_=sd[:]
                    )
                    xns = []
                    for ti in range(4):
                        tt = ttg * 4 + ti
                        xn = ph1s.tile([128, C], BF, tag=f"xn{ti}", name=f"xn{ti}")
                        nc.vector.tensor_scalar(
                            out=xn[:], in0=xt[:, ti, :],
                            scalar1=mvall[:, tt, 0:1],
                            scalar2=rstd1[:, tt : tt + 1],
                            op0=ALU.subtract, op1=ALU.mult,
                        )
                        xns.append(xn)
                    for half in range(2):
                        ptb = [
                            ph1t.tile([128, 512], F32, tag=f"ptb{t}", name=f"ptb{t}")
                            for t in range(3)
                        ]
                        for ti in range(4):
                            for ci, ct in enumerate(range(3 * half, 3 * half + 3)):
                                nc.tensor.matmul(
                                    ptb[ci][:, ti * 128 : (ti + 1) * 128],
                                    xns[ti][:, ct * 128 : (ct + 1) * 128],
                                    ident,
                                    start=(ti == 0), stop=(ti == 3),
                                    skip_group_check=True,
                                )
                        for ci, ct in enumerate(range(3 * half, 3 * half + 3)):
                            kt, r = ct // 2, ct % 2
                            nc.scalar.copy(
                                out=xnT8[kt][:, r, ttg * 512 : (ttg + 1) * 512],
                                in_=ptb[ci][:],
                            )
                            nc.vector.tensor_copy(
                                xnTq8[kt][:, r, ttg * 128 : (ttg + 1) * 128],
                                ptb[ci][:, 384:512],
                            )
                    # kT chunk for this token group (all 6 feature blocks)
                    for f in range(CT):
                        ps = ph2k.tile([128, 512], F32, tag="pqk", name="pk")
                        for kt in range(KT):
                            nc.tensor.matmul(
                                ps[:], wk8[kt][:, :, f * 128 : (f + 1) * 128],
                                xnT8[kt][:, :, ttg * 512 : (ttg + 1) * 512],
                                start=(kt == 0), stop=(kt == KT - 1),
                                perf_mode=DR,
                            )
                        nc.scalar.copy(
                            out=kT[f][:, ttg * 512 : (ttg + 1) * 512], in_=ps[:]
                        )
        # =============== phase 2 + 3: Q^T, V2, attention ==================
        with (
            pool("attn_sb", bufs=1) as attnp,
            pool("p4p", bufs=4) as p4p,
            pool("rrp", bufs=2) as rrp,
        ):

            with pool("ph2q", bufs=2, space="PSUM") as ph2q:
                # qT[f] = (Wq[:, f].T @ xnTq) + bq32
                for f in range(CT):
                    ps = ph2q.tile([128, NQ], F32, tag="pq", name="pq")
                    for kt in range(KT):
                        nc.tensor.matmul(
                            ps[:], wq8[kt][:, :, f * 128 : (f + 1) * 128],
                            xnTq8[kt][:], start=(kt == 0), stop=(kt == KT - 1),
                            perf_mode=DR,
                        )
                    nc.vector.tensor_scalar(
                        out=qT[f][:], in0=ps[:], scalar1=bq32[:, f : f + 1],
                        scalar2=None, op0=ALU.add,
                    )

            # ---- attention: 6 head pairs x 8 key pairs ----
            # PSUM: ps 2x2 banks + pavden 2 banks + pv 2 banks = 8
            with (
                pool("ph3s", bufs=2, space="PSUM") as ph3s,
                pool("ph3a", bufs=1, space="PSUM") as ph3a,
                pool("ph2v", bufs=1, space="PSUM") as ph2v,
            ):
                def build_v2(j):
                    for r in range(2):
                        tt = 2 * j + r
                        pv = ph2v.tile([128, C], F32, tag="pv", name="pv")
                        for lo, hi in ((0, 512), (512, 768)):
                            for kt in range(KT):
                                nc.tensor.matmul(
                                    pv[:, lo:hi],
                                    xnT8[kt][:, :, tt * 128 : (tt + 1) * 128],
                                    wv8[kt][:, :, lo:hi],
                                    start=(kt == 0), stop=(kt == KT - 1),
                                    perf_mode=DR,
                                )
                        dst = V2[j][:, r, :].rearrange("p (h e) -> p h e", e=128)[
                            :, :, 0:HD
                        ]
                        srcv = pv[:].rearrange("p (h e) -> p h e", e=HD)
                        nc.vector.tensor_copy(dst, srcv)

                def emit_proj_round(kt):
                    for qt in range(4):
                        for lo, hi in ((0, 512), (512, 768)):
                            ppb = ph2v.tile([128, C], F32, tag="pv", name="pp")
                            pp = ppb[:, 0:512]
                            nc.tensor.matmul(
                                pp[:, 0 : hi - lo],
                                yT[kt][:, :, qt * 128 : (qt + 1) * 128],
                                wp8[kt][:, :, lo:hi],
                                perf_mode=DR,
                            )
                            if kt == 0:
                                nc.vector.tensor_tensor(
                                    out=x2[:, qt, lo:hi], in0=pp[:, 0 : hi - lo],
                                    in1=xo32[:, qt, lo:hi], op=ALU.add,
                                )
                            else:
                                nc.vector.tensor_tensor(
                                    out=x2[:, qt, lo:hi], in0=x2[:, qt, lo:hi],
                                    in1=pp[:, 0 : hi - lo], op=ALU.add,
                                )

                for pi in range(6):  # head pair index
                    pavden = [
                        ph3a.tile([128, NQ], F32, tag=f"pav{i}", name=f"pav{i}")
                        for i in range(2)
                    ]
                    items = []
                    for j in range(KPP):
                        cs = 128 * (j // 2)
                        for ci, (c0, c1) in enumerate(_chunks(cs)):
                            items.append((j, ci, c0, c1))

                    def emit_score(idx):
                        j, ci, c0, c1 = items[idx]
                        if pi == 0 and ci == 0:
                            build_v2(j)  # just-in-time, overlaps exp
                        cs = 128 * (j // 2)
                        w = c1 - c0
                        psb = ph3s.tile([128, 2, 2, 256], F32, tag="ps", name="ps")
                        ps = psb[:, :, :, 0:w] if w < 256 else psb[:]
                        has_mask = c0 == cs
                        for hi in range(2):
                            for r in range(2):
                                kp = 2 * j + r
                                nc.tensor.matmul(
                                    ps[:, hi, r, :],
                                    kT[pi][
                                        hi * 64 : hi * 64 + 64,
                                        kp * 128 : (kp + 1) * 128,
                                    ],
                                    qT[pi][hi * 64 : hi * 64 + 64, c0:c1],
                                    start=(r == 0),
                                    stop=(hi == 1 and r == 1),
                                    skip_group_check=True,
                                )
                        p4b = p4p.tile([128, 2, 2, 256], F8, tag="p4", name="p4")
                        p4 = p4b[:, :, :, 0:w] if w < 256 else p4b[:]
                        nc.scalar.activation(
                            out=p4[:], in_=ps[:], func=AF.Exp,
                            scale=0.125 / (WS * WS),
                        )
                        if has_mask:
                            nc.vector.tensor_tensor(
                                out=p4[:, :, :, 0:128], in0=p4[:, :, :, 0:128],
                                in1=mrep[:, j, :, :, :], op=ALU.mult,
                            )
                        return p4

                    def emit_av(idx, p4):
                        j, ci, c0, c1 = items[idx]
                        for hi in range(2):
                            h = 2 * pi + hi
                            nc.tensor.matmul(
                                pavden[hi][:, c0:c1],
                                V2[j][:, :, h * 128 : (h + 1) * 128],
                                p4[:, hi, :, :],
                                start=(idx == 0),
                                stop=(idx == len(items) - 1),
                                perf_mode=DR,
                                skip_group_check=True,
                            )

                    prev = None
                    for idx in range(len(items)):
                        p4 = emit_score(idx)
                        if prev is not None:
                            emit_av(idx - 1, prev)
                        prev = p4
                    emit_av(len(items) - 1, prev)
                    # normalize: rows 64:128 of pavden hold den * 64
                    rr = rrp.tile([128, NQ], F32, tag="rr", name="rr")
                    rr2 = rrp.tile([128, NQ], F32, tag="rr2", name="rr2")
                    nc.vector.tensor_copy(rr2[0:64, :], pavden[0][64:128, :])
                    nc.vector.tensor_copy(rr2[64:128, :], pavden[1][64:128, :])
                    nc.vector.reciprocal_approx_fast(out=rr[:], in_=rr2[:])
                    kt, r = pi // 2, pi % 2
                    nc.vector.tensor_tensor(
                        out=yT[kt][0:64, r, :], in0=pavden[0][0:64, :],
                        in1=rr[0:64, :], op=ALU.mult,
                    )
                    nc.vector.tensor_tensor(
                        out=yT[kt][64:128, r, :], in0=pavden[1][0:64, :],
                        in1=rr[64:128, :], op=ALU.mult,
                    )
                    if pi == 5:
                        for kt_ in range(KT):
                            emit_proj_round(kt_)

        attn2_cm.__exit__(None, None, None)

        # =============== phase 4: proj + residual + LN2 ===============
        with pool("mlp_sb", bufs=1) as mlpp:
            wfc2_t = mlpp.tile([128, FT, C], BF, tag="wfc2t", name="wfc2t")
            wfc2 = [wfc2_t[:, t, :] for t in range(FT)]
            nc.sync.dma_start(
                wfc2_t[:], d["d_wfc2"][:].rearrange("(t p) c -> p t c", p=128)
            )
            xn2T = [mlpp.tile([128, NQ], BF, tag=f"xn2T{t}", name=f"xn2T{t}") for t in range(CT)]
            hT = [mlpp.tile([128, NQ], BF, tag=f"hT{t}", name=f"hT{t}") for t in range(FT)]
            mv2 = mlpp.tile([128, 4, 2], F32, tag="mv2", name="mv2")
            rstd2 = mlpp.tile([128, 4], F32, tag="rstd2", name="rstd2")

            with (
                pool("ph4t", bufs=2, space="PSUM") as ph4t,
                pool("ph4s", bufs=4) as ph4s,
            ):
                for qt in range(4):
                    stats = ph4s.tile([128, 2, 6], F32, tag="bns2", name="bns2")
                    xg = x2[:, qt, :].rearrange("p (a b) -> p a b", b=384)
                    for a in range(2):
                        nc.vector.bn_stats(out=stats[:, a, :], in_=xg[:, a, :])
                    nc.vector.bn_aggr(out=mv2[:, qt, :], in_=stats[:])
                    sd2 = ph4s.tile([128, 1], F32, tag="sd2", name="sd2")
                    nc.scalar.activation(
                        out=sd2[:], in_=mv2[:, qt, 1:2], func=AF.Sqrt, bias=eps[:]
                    )
                    nc.vector.reciprocal(out=rstd2[:, qt : qt + 1], in_=sd2[:])
                for qt in range(4):
                    xn2 = ph4s.tile([128, C], BF, tag="xn2", name="xn2")
                    nc.vector.tensor_scalar(
                        out=xn2[:], in0=x2[:, qt, :], scalar1=mv2[:, qt, 0:1],
                        scalar2=rstd2[:, qt : qt + 1],
                        op0=ALU.subtract, op1=ALU.mult,
                    )
                    nc.vector.tensor_tensor(
                        out=x2[:, qt, :], in0=x2[:, qt, :], in1=bfc2b[:],
                        op=ALU.add,
                    )
                    ptb2 = ph4t.tile([128, C], F32, tag="ptb2", name="ptb2")
                    for ct in range(CT):
                        nc.tensor.matmul(
                            ptb2[:, ct * 128 : (ct + 1) * 128],
                            xn2[:, ct * 128 : (ct + 1) * 128], ident,
                            start=(ct in (0, 4)), stop=(ct in (3, 5)),
                            skip_group_check=True,
                        )
                    for ct in range(CT):
                        nc.scalar.copy(
                            out=xn2T[ct][:, qt * 128 : (qt + 1) * 128],
                            in_=ptb2[:, ct * 128 : (ct + 1) * 128],
                        )

            # =============== phase 5: fc + gelu (fp8 out) =================
            with pool("ph5p", bufs=3, space="PSUM") as ph5p:
                for f in range(FT):
                    ph_ = ph5p.tile([128, NQ], F32, tag="ph5", name="ph5")
                    for ct in range(CT):
                        nc.tensor.matmul(
                            ph_[:],
                            wfc[ct][:, f * 128 : (f + 1) * 128],
                            xn2T[ct][:],
                            start=(ct == 0), stop=(ct == CT - 1),
                        )
                    nc.scalar.activation(
                        out=hT[f][:], in_=ph_[:],
                        func=AF.Gelu_apprx_tanh,
                        bias=bfcc[:, f : f + 1],
                    )

            # =============== phase 7: fc2 + residual + out ================
            with (
                pool("ph7p", bufs=2, space="PSUM") as ph7p,
                pool("ph7s", bufs=2) as ph7s,
            ):
                for qt in range(4):
                    po = ph7p.tile([128, C], F32, tag="po", name="po")
                    for lo, hi in ((0, 512), (512, 768)):
                        for kt in range(FT):
                            nc.tensor.matmul(
                                po[:, lo:hi],
                                hT[kt][:, qt * 128 : (qt + 1) * 128],
                                wfc2[kt][:, lo:hi],
                                start=(kt == 0), stop=(kt == FT - 1),
                            )
                    ot = ph7s.tile([128, C], F32, tag="ot", name="ot")
                    nc.vector.tensor_tensor(
                        out=ot[:], in0=po[:], in1=x2[:, qt, :], op=ALU.add
                    )
                    nc.sync.dma_start(
                        d["d_out"][qt * 128 : (qt + 1) * 128, :], ot[:]
                    )


# ---------------------------------------------------------------------------
# Host-side wrapper
# ---------------------------------------------------------------------------
_PROGRAM = None


def _get_program():
    global _PROGRAM
    if _PROGRAM is None:
        _PROGRAM = build_program()
    return _PROGRAM


def make_in_maps(x, ln1_g, ln1_b, W_attn, b_attn, W_proj, b_proj,
                 ln2_g, ln2_b, W_fc, b_fc, W_fc2, b_fc2):
    x = np.asarray(x, np.float32)
    f = np.float32
    ln1_g, ln1_b = np.asarray(ln1_g, f), np.asarray(ln1_b, f)
    ln2_g, ln2_b = np.asarray(ln2_g, f), np.asarray(ln2_b, f)
    W_attn, b_attn = np.asarray(W_attn, f), np.asarray(b_attn, f)
    W_proj, b_proj = np.asarray(W_proj, f), np.asarray(b_proj, f)
    W_fc, b_fc = np.asarray(W_fc, f), np.asarray(b_fc, f)
    W_fc2, b_fc2 = np.asarray(W_fc2, f), np.asarray(b_fc2, f)

    # LN1 gain folded into W_attn rows; LN1 bias folded into the qkv biases.
    Wg = ln1_g[:, None] * W_attn
    bqkv = ln1_b @ W_attn + b_attn
    bq_eff = bqkv[0:C]
    bv_eff = bqkv[2 * C : 3 * C]
    # K bias dropped (softmax-invariant); V bias folded into b_proj.
    bp_eff = b_proj + bv_eff @ W_proj
    # LN2 gain/bias folded into W_fc / b_fc.
    Wfcg = ln2_g[:, None] * W_fc
    bfc_eff = ln2_b @ W_fc + b_fc

    ident = np.eye(128, dtype=np.float32)
    idents = np.concatenate([ident, -MBIG * ident], axis=1).astype(BF16)
    cn = np.concatenate([
        (WS * bq_eff).reshape(CT, 128).T,
        bfc_eff.reshape(FT, 128).T,
        np.broadcast_to(WS * b_fc2, (128, C)),
    ], axis=1).astype(np.float32).copy()

    shared = {
        "wq": pack_dr(Wg[:, 0:C]),
        "wk": pack_dr(Wg[:, C : 2 * C]),
        "wv": pack_dr(Wg[:, 2 * C : 3 * C]),
        "wp": pack_dr(W_proj),
        "wfc": np.asarray(Wfcg, BF16),
        "wfc2": np.asarray(WS * W_fc2, BF16),
        "idents": idents,
        "consts": cn,
    }
    in_maps, layouts = [], []
    for core in range(8):
        b, g = core // 4, core % 4
        qtiles, perm = core_layout(g)
        idx = np.concatenate([np.arange(t * 128, (t + 1) * 128) for t in perm])
        own = np.concatenate([np.arange(t * 128, (t + 1) * 128) for t in qtiles])
        m = dict(shared)
        m["x_perm"] = np.ascontiguousarray(x[b][idx]).astype(BF16)
        m["x_own32"] = (WS * (x[b][own] + bp_eff)).astype(BF16)
        m["maskrep"] = core_maskneg(qtiles, perm)
        in_maps.append(m)
        layouts.append((b, own))
    return in_maps, layouts


def unshard(results, layouts):
    out = np.empty((B, T, C), np.float32)
    for r, (b, own) in zip(results, layouts):
        out[b][own] = r["out"] * (1.0 / WS)
    return out


def kernel(**inputs):
    from concourse.bass_utils import run_bass_kernel_spmd

    nc = _get_program()
    in_maps, layouts = make_in_maps(**inputs)
    res = run_bass_kernel_spmd(nc, in_maps, core_ids=list(range(8)))
    return unshard(res.results, layouts)


# revision 40
# speedup vs baseline: 1.0327x; 1.0327x over previous
"""GPT-2 block (B=2, T=2048, C=768, H=12) on 8 Trainium2 NeuronCores.

Sharding: data-parallel over batch (2) x 4-way query-tile split per batch.
Each core computes K/V for its full batch (no on-chip collectives) and runs
attention + MLP for 4 of the 16 query tiles, interleaved {g, 7-g, 8+g, 15-g}
so the causal-attention work is identical across cores.  The SPMD program is
uniform: per-core differences are pushed into the data via a k-tile
permutation of the sequence plus per-core causal masks.

Precision/layout highlights (v2):
- Every GEMM is fp8 e4m3 DoubleRow (2 k-rows/cycle): QKV, proj, fc, fc2 and
  the attention A*V.  Weights are pre-scaled by 32 on the host; LayerNorm
  gains/biases are folded into the adjacent weights/biases host-side, the
  K-bias is dropped (softmax-invariant) and the V-bias rides b_proj.
- Attention: scores are computed transposed (S^T [k,q]) in kp-PAIRS so the
  exp() output is directly the fp8 DoubleRow moving operand of the fused
  A*V matmul whose stationary is [V | 64 ones-columns] - the softmax
  denominator lands in psum rows 64:128, giving one full-partition
  reciprocal + one multiply per head pair.
- Causal masks are applied on the PE: a matmul with stationary -2^20*I and
  per-core (1-mask) moving data accumulates -2^20 into masked score slots
  before exp (data-driven masking, uniform SPMD program).
- The attention residual carries a 32x scale (x_own pre-scaled on host);
  LayerNorm is scale-invariant, so the scale rides through to the output,
  which the host divides by 32.
"""

import sys

sys.path.insert(0, "/opt/trn_rl_repo")

import numpy as np
import ml_dtypes

import bass_rust
import concourse.bass as bass
import concourse.bacc as bacc
import concourse.tile as tile
from concourse import mybir
from concourse.vector_clock import ScopedClock

BF16 = ml_dtypes.bfloat16
F32 = mybir.dt.float32
BF = mybir.dt.bfloat16
F8 = mybir.dt.float8e4
NP_F8 = mybir.dt.np(F8)

B, T, C, H = 2, 2048, 768, 12
HD = C // H  # 64
DFF = 4 * C  # 3072
TT = T // 128  # 16 token tiles
CT = C // 128  # 6 feature tiles
KT = C // 256  # 3 DoubleRow k-tiles over C
KT2 = DFF // 256  # 12 DoubleRow k-tiles over DFF
FT = DFF // 128  # 24
KPP = TT // 2  # 8 key-tile pairs
QPOS = (3, 7, 11, 15)  # fixed positions of this core's query tiles
NQ = 512  # queries per core
WS = 32.0  # fp8 weight pre-scale
MBIG = float(2**20)  # mask magnitude (scaled to -128 at exp input)
AF = mybir.ActivationFunctionType
ALU = mybir.AluOpType
DR = mybir.MatmulPerfMode.DoubleRow

# ---------------------------------------------------------------------------
# Tile exit-drain fix: the final SP drain carries one wait per live logical
# processor, but TRN2 ISA instructions hold at most 1 embedded sync wait in
# this toolchain. Split the waits across a chain of SP drains.
# ---------------------------------------------------------------------------
_MAX_WAITS = 1


def _drain_and_barrier(self, tick_clock, wait_clock):
    drain_inst = self.nc.sync.drain()
    wait_clock.add_sem_waits(
        drain_inst.ins, ScopedClock({None: tick_clock.global_clock})
    )
    si = drain_inst.ins.sync_info
    if si is not None and len(si.on_wait) > _MAX_WAITS:
        waits = list(si.on_wait)
        drain_inst.ins.sync_info = bass_rust.SyncInfo(
            on_wait=waits[:_MAX_WAITS], on_update=list(si.on_update)
        )
        rest = waits[_MAX_WAITS:]
        for i in range(0, len(rest), _MAX_WAITS):
            extra = self.nc.sync.drain()
            extra.ins.sync_info = bass_rust.SyncInfo(
                on_wait=rest[i : i + _MAX_WAITS], on_update=[]
            )
    self.nc.all_engine_barrier()
    assert self.sems is not None
    popped = self.nc._tile_sem_poison_stack.pop()
    assert popped is self._sem_poison
    self.nc.clear_and_free_semaphores(list(self.sems.allocated().values()))
    self.nc.all_engine_barrier()


tile.TileContext._drain_and_barrier = _drain_and_barrier


# ---------------------------------------------------------------------------
# Per-core sharding layout (host side)
# ---------------------------------------------------------------------------
def core_layout(g):
    """For group index g (0..3): (qtiles sorted, perm) with the core's query
    tiles at positions QPOS and every tile's causal prefix placed before it."""
    qtiles = sorted([g, 7 - g, 8 + g, 15 - g])
    posmap = dict(zip(QPOS, qtiles))
    rest = iter([t for t in range(TT) if t not in qtiles])
    perm = [posmap[p] if p in posmap else next(rest) for p in range(TT)]
    for j, a in enumerate(qtiles):
        assert set(range(a + 1)) <= set(perm[: QPOS[j] + 1]), (g, j, perm)
    return qtiles, perm


def core_maskneg(qtiles, perm):
    """maskrep[j, :, hi, r, :] = causal mask for k-position kp=2j+r against
    query tile bi=kp//4 (first in-suffix 128-query block), replicated over
    the head (hi) axis. Multiplied into the exp output on the DVE."""
    mrep = np.zeros((KPP, 128, 2, 2, 128), dtype=BF16)
    for kp in range(TT):
        tk = perm[kp] * 128 + np.arange(128)[:, None]
        a = qtiles[kp // 4]
        tq = a * 128 + np.arange(128)[None, :]
        m = (tk <= tq).astype(np.float32)
        mrep[kp // 2, :, 0, kp % 2, :] = m
        mrep[kp // 2, :, 1, kp % 2, :] = m
    return mrep


def pack_dr(W):
    """[K, N] fp32 -> DoubleRow-paired fp8 [K/256, 128, 2, N], pre-scaled.
    Logical k = 256*kt + 128*r + p."""
    K, N = W.shape
    Wp = (np.asarray(W, np.float32) * WS).reshape(K // 256, 2, 128, N)
    return np.ascontiguousarray(Wp.transpose(0, 2, 1, 3)).astype(NP_F8)


def _chunks(cs):
    """Query-column chunks (<=256 wide) covering [cs, NQ)."""
    if NQ - cs > 256:
        return [(cs, cs + 256), (cs + 256, NQ)]
    return [(cs, NQ)]


# ---------------------------------------------------------------------------
# The Bass program (identical for all 8 cores)
# ---------------------------------------------------------------------------
def build_program():
    nc = bacc.Bacc("TRN2")

    d_x = nc.dram_tensor("x_perm", [T, C], BF, kind="ExternalInput")
    d_xo = nc.dram_tensor("x_own32", [NQ, C], BF, kind="ExternalInput")
    d_mneg = nc.dram_tensor("maskrep", [KPP, 128, 2, 2, 128], BF, kind="ExternalInput")
    d_wq = nc.dram_tensor("wq", [KT, 128, 2, C], F8, kind="ExternalInput")
    d_wk = nc.dram_tensor("wk", [KT, 128, 2, C], F8, kind="ExternalInput")
    d_wv = nc.dram_tensor("wv", [KT, 128, 2, C], F8, kind="ExternalInput")
    d_wp = nc.dram_tensor("wp", [KT, 128, 2, C], F8, kind="ExternalInput")
    d_wfc = nc.dram_tensor("wfc", [C, DFF], BF, kind="ExternalInput")
    d_wfc2 = nc.dram_tensor("wfc2", [DFF, C], BF, kind="ExternalInput")
    # [ident | -2^20*ident]
    d_idents = nc.dram_tensor("idents", [128, 256], BF, kind="ExternalInput")
    # [bq32 (CT) | bfcc (FT) | bfc2b*32 (C)]
    d_cn = nc.dram_tensor("consts", [128, CT + FT + C], F32, kind="ExternalInput")
    d_out = nc.dram_tensor("out", [NQ, C], F32, kind="ExternalOutput")

    with tile.TileContext(nc) as tc:
        _body(nc, tc, locals())
    nc.compile()
    return nc


def _body(nc, tc, d):
    def pool(name, **kw):
        return tc.tile_pool(name=name, **kw)

    with (
        pool("const", bufs=1) as constp,
        pool("pers", bufs=1) as pers,
        pool("small", bufs=6) as small,
    ):
        # ---- constants ---------------------------------------------------
        idents = constp.tile([128, 2, 128], BF)
        nc.scalar.dma_start(
            idents[:], d["d_idents"][:].rearrange("p (a b) -> p a b", b=128)
        )
        ident, identn = idents[:, 0, :], idents[:, 1, :]
        eps = constp.tile([128, 1], F32)
        nc.vector.memset(eps[:], 1e-5)
        cn = constp.tile([128, CT + FT + C], F32)
        nc.scalar.dma_start(cn[:], d["d_cn"][:])
        bq32 = cn[:, 0:CT]
        bfcc = cn[:, CT : CT + FT]
        bfc2b = cn[:, CT + FT :]
        mrep = constp.tile([128, KPP, 2, 2, 128], BF)
        nc.gpsimd.dma_start(mrep[:], d["d_mneg"][:].rearrange("j p a r q -> p j a r q"))

        # ---- persistent tiles --------------------------------------------
        wp8 = [pers.tile([128, 2, C], F8, tag=f"wp{t}", name=f"wp{t}") for t in range(KT)]
        wfc_t = pers.tile([128, CT, DFF], BF, tag="wfct", name="wfct")
        wfc = [wfc_t[:, t, :] for t in range(CT)]
        xo32 = pers.tile([128, 4, C], BF, tag="xo", name="xo")
        kT = [pers.tile([128, T], BF, tag=f"kT{t}", name=f"kT{t}") for t in range(CT)]
        qT = [pers.tile([128, NQ], BF, tag=f"qT{t}", name=f"qT{t}") for t in range(CT)]
        xnT8 = [pers.tile([128, 2, T], F8, tag=f"xnT{t}", name=f"xnT{t}") for t in range(KT)]
        xnTq8 = [pers.tile([128, 2, NQ], F8, tag=f"xnTq{t}", name=f"xnTq{t}") for t in range(KT)]
        yT = [pers.tile([128, 2, NQ], F8, tag=f"yT{t}", name=f"yT{t}") for t in range(KT)]
        mvall = pers.tile([128, TT, 2], F32, tag="mvall", name="mvall")
        x2 = pers.tile([128, 4, C], F32, tag="x2", name="x2")
        rstd1 = pers.tile([128, TT], F32, tag="rstd1", name="rstd1")

        # ---- input DMAs (issued up-front; per-queue FIFO sets priority) --
        for t in range(KT):
            nc.scalar.dma_start(wp8[t][:], d["d_wp"][t])
        nc.gpsimd.dma_start(
            xo32[:], d["d_xo"][:].rearrange("(t p) c -> p t c", p=128)
        )

        # =============== phase 1: LN1 + transpose, feature-major fp8 ======
        wq8 = [pers.tile([128, 2, C], F8, tag=f"wq{t}", name=f"wq{t}") for t in range(KT)]
        PH1_MARKER = 1
        wk8 = [pers.tile([128, 2, C], F8, tag=f"wk{t}", name=f"wk{t}") for t in range(KT)]
        wv8 = [pers.tile([128, 2, C], F8, tag=f"wv{t}", name=f"wv{t}") for t in range(KT)]
        for t in range(KT):
            nc.scalar.dma_start(wk8[t][:], d["d_wk"][t])
        for t in range(KT):
            nc.scalar.dma_start(wv8[t][:], d["d_wv"][t])
        for t in range(KT):
            nc.scalar.dma_start(wq8[t][:], d["d_wq"][t])
        attn2_cm = tc.tile_pool(name="attn2", bufs=1)
        attn2 = attn2_cm.__enter__()
        # V2[j]: DoubleRow stationary over key pairs (tiles 2j, 2j+1):
        # per head 128 cols = [64 V | 64 ones*WS]
        V2 = [
            attn2.tile([128, 2, H * 128], F8, tag=f"V2{j}", name=f"V2{j}")
            for j in range(KPP)
        ]
        for j in range(KPP):
            v4 = V2[j][:].rearrange("p r (h e) -> p r h e", e=128)
            nc.gpsimd.memset(v4[:, :, :, HD:], WS)
        with (
            pool("ph1x", bufs=3) as ph1x,
            pool("ph1s", bufs=2) as ph1s,
        ):
            nc.gpsimd.dma_start(
                wfc_t[:], d["d_wfc"][:].rearrange("(t p) c -> p t c", p=128)
            )


            with (
                pool("ph1t", bufs=1, space="PSUM") as ph1t,
                pool("ph2k", bufs=2, space="PSUM") as ph2k,
            ):
                # PE warm-up: keep the HAM activity monitor at full clock
                warm = ph2k.tile([128, 512], F32, tag="pqk", name="warm")
                for _ in range(128):
                    nc.tensor.matmul(warm[:, 0:128], ident, ident)

                for ttg in range(4):
                    xt = ph1x.tile([128, 4, C], BF, tag="xt", name="xt")
                    nc.sync.dma_start(
                        xt[:],
                        d["d_x"][ttg * 512 : (ttg + 1) * 512, :].rearrange(
                            "(t p) c -> p t c", p=128
                        ),
                    )
                    # LN stats for the 4 tiles of this group
                    for ti in range(4):
                        tt = ttg * 4 + ti
                        stats = ph1s.tile([128, 2, 6], F32, tag="bnst", name="bnst")
                        xg = xt[:, ti, :].rearrange("p (a b) -> p a b", b=384)
                        for a in range(2):
                            nc.vector.bn_stats(out=stats[:, a, :], in_=xg[:, a, :])
                        nc.vector.bn_aggr(out=mvall[:, tt, :], in_=stats[:])
                    sd = ph1s.tile([128, 4], F32, tag="sd", name="sd")
                    nc.scalar.activation(
                        out=sd[:],
                        in_=mvall[:, 4 * ttg : 4 * ttg + 4, 1],
                        func=AF.Sqrt,
                        bias=eps[:],
                    )
                    nc.vector.reciprocal(
                        out=rstd1[:, 4 * ttg : 4 * ttg + 4], in_=sd[:]
                    )
                    xns = []
                    for ti in range(4):
                        tt = ttg * 4 + ti
                        xn = ph1s.tile([128, C], BF, tag=f"xn{ti}", name=f"xn{ti}")
                        nc.vector.tensor_scalar(
                            out=xn[:], in0=xt[:, ti, :],
                            scalar1=mvall[:, tt, 0:1],
                            scalar2=rstd1[:, tt : tt + 1],
                            op0=ALU.subtract, op1=ALU.mult,
                        )
                        xns.append(xn)
                    for half in range(2):
                        ptb = [
                            ph1t.tile([128, 512], F32, tag=f"ptb{t}", name=f"ptb{t}")
                            for t in range(3)
                        ]
                        for ti in range(4):
                            for ci, ct in enumerate(range(3 * half, 3 * half + 3)):
                                nc.tensor.matmul(
                                    ptb[ci][:, ti * 128 : (ti + 1) * 128],
                                    xns[ti][:, ct * 128 : (ct + 1) * 128],
                                    ident,
                                    start=(ti == 0), stop=(ti == 3),
                                    skip_group_check=True,
                                )
                        for ci, ct in enumerate(range(3 * half, 3 * half + 3)):
                            kt, r = ct // 2, ct % 2
                            nc.scalar.copy(
                                out=xnT8[kt][:, r, ttg * 512 : (ttg + 1) * 512],
                                in_=ptb[ci][:],
                            )
                            nc.vector.tensor_copy(
                                xnTq8[kt][:, r, ttg * 128 : (ttg + 1) * 128],
                                ptb[ci][:, 384:512],
                            )
                    # kT chunk for this token group (all 6 feature blocks)
                    for f in range(CT):
                        ps = ph2k.tile([128, 512], F32, tag="pqk", name="pk")
                        for kt in range(KT):
                            nc.tensor.matmul(
                                ps[:], wk8[kt][:, :, f * 128 : (f + 1) * 128],
                                xnT8[kt][:, :, ttg * 512 : (ttg + 1) * 512],
                                start=(kt == 0), stop=(kt == KT - 1),
                                perf_mode=DR,
                            )
                        nc.scalar.copy(
                            out=kT[f][:, ttg * 512 : (ttg + 1) * 512], in_=ps[:]
                        )
        # =============== phase 2 + 3: Q^T, V2, attention ==================
        with (
            pool("attn_sb", bufs=1) as attnp,
            pool("p4p", bufs=4) as p4p,
            pool("rrp", bufs=2) as rrp,
        ):

            with pool("ph2q", bufs=2, space="PSUM") as ph2q:
                # qT[f] = (Wq[:, f].T @ xnTq) + bq32
                for f in range(CT):
                    ps = ph2q.tile([128, NQ], F32, tag="pq", name="pq")
                    for kt in range(KT):
                        nc.tensor.matmul(
                            ps[:], wq8[kt][:, :, f * 128 : (f + 1) * 128],
                            xnTq8[kt][:], start=(kt == 0), stop=(kt == KT - 1),
                            perf_mode=DR,
                        )
                    nc.vector.tensor_scalar(
                        out=qT[f][:], in0=ps[:], scalar1=bq32[:, f : f + 1],
                        scalar2=None, op0=ALU.add,
                    )

            # ---- attention: 6 head pairs x 8 key pairs ----
            # PSUM: ps 2x2 banks + pavden 2 banks + pv 2 banks = 8
            with (
                pool("ph3s", bufs=2, space="PSUM") as ph3s,
                pool("ph3a", bufs=1, space="PSUM") as ph3a,
                pool("ph2v", bufs=1, space="PSUM") as ph2v,
            ):
                def build_v2(j):
                    for r in range(2):
                        tt = 2 * j + r
                        pv = ph2v.tile([128, C], F32, tag="pv", name="pv")
                        for lo, hi in ((0, 512), (512, 768)):
                            for kt in range(KT):
                                nc.tensor.matmul(
                                    pv[:, lo:hi],
                                    xnT8[kt][:, :, tt * 128 : (tt + 1) * 128],
                                    wv8[kt][:, :, lo:hi],
                                    start=(kt == 0), stop=(kt == KT - 1),
                                    perf_mode=DR,
                                )
                        dst = V2[j][:, r, :].rearrange("p (h e) -> p h e", e=128)[
                            :, :, 0:HD
                        ]
                        srcv = pv[:].rearrange("p (h e) -> p h e", e=HD)
                        nc.vector.tensor_copy(dst, srcv)

                for pi in range(6):  # head pair index
                    pavden = [
                        ph3a.tile([128, NQ], F32, tag=f"pav{i}", name=f"pav{i}")
                        for i in range(2)
                    ]
                    items = []
                    for j in range(KPP):
                        cs = 128 * (j // 2)
                        for ci, (c0, c1) in enumerate(_chunks(cs)):
                            items.append((j, ci, c0, c1))

                    def emit_score(idx):
                        j, ci, c0, c1 = items[idx]
                        if pi == 0 and ci == 0:
                            build_v2(j)  # just-in-time, overlaps exp
                        cs = 128 * (j // 2)
                        w = c1 - c0
                        psb = ph3s.tile([128, 2, 2, 256], F32, tag="ps", name="ps")
                        ps = psb[:, :, :, 0:w] if w < 256 else psb[:]
                        has_mask = c0 == cs
                        for hi in range(2):
                            for r in range(2):
                                kp = 2 * j + r
                                nc.tensor.matmul(
                                    ps[:, hi, r, :],
                                    kT[pi][
                                        hi * 64 : hi * 64 + 64,
                                        kp * 128 : (kp + 1) * 128,
                                    ],
                                    qT[pi][hi * 64 : hi * 64 + 64, c0:c1],
                                    start=(r == 0),
                                    stop=(hi == 1 and r == 1),
                                    skip_group_check=True,
                                )
                        p4b = p4p.tile([128, 2, 2, 256], F8, tag="p4", name="p4")
                        p4 = p4b[:, :, :, 0:w] if w < 256 else p4b[:]
                        nc.scalar.activation(
                            out=p4[:], in_=ps[:], func=AF.Exp,
                            scale=0.125 / (WS * WS),
                        )
                        if has_mask:
                            nc.vector.tensor_tensor(
                                out=p4[:, :, :, 0:128], in0=p4[:, :, :, 0:128],
                                in1=mrep[:, j, :, :, :], op=ALU.mult,
                            )
                        return p4

                    def emit_av(idx, p4):
                        j, ci, c0, c1 = items[idx]
                        for hi in range(2):
                            h = 2 * pi + hi
                            nc.tensor.matmul(
                                pavden[hi][:, c0:c1],
                                V2[j][:, :, h * 128 : (h + 1) * 128],
                                p4[:, hi, :, :],
                                start=(idx == 0),
                                stop=(idx == len(items) - 1),
                                perf_mode=DR,
                                skip_group_check=True,
                            )

                    prev = None
                    for idx in range(len(items)):
                        p4 = emit_score(idx)
                        if prev is not None:
                            emit_av(idx - 1, prev)
                        prev = p4
                    emit_av(len(items) - 1, prev)
                    # normalize: rows 64:128 of pavden hold den * 64
                    rr = rrp.tile([128, NQ], F32, tag="rr", name="rr")
                    rr2 = rrp.tile([128, NQ], F32, tag="rr2", name="rr2")
                    nc.vector.tensor_copy(rr2[0:64, :], pavden[0][64:128, :])
                    nc.vector.tensor_copy(rr2[64:128, :], pavden[1][64:128, :])
                    nc.vector.reciprocal_approx_fast(out=rr[:], in_=rr2[:])
                    kt, r = pi // 2, pi % 2
                    nc.vector.tensor_tensor(
                        out=yT[kt][0:64, r, :], in0=pavden[0][0:64, :],
                        in1=rr[0:64, :], op=ALU.mult,
                    )
                    nc.vector.tensor_tensor(
                        out=yT[kt][64:128, r, :], in0=pavden[1][0:64, :],
                        in1=rr[64:128, :], op=ALU.mult,
                    )

        attn2_cm.__exit__(None, None, None)

        # =============== phase 4: proj + residual + LN2 ===============
        with pool("mlp_sb", bufs=1) as mlpp:
            wfc2_t = mlpp.tile([128, FT, C], BF, tag="wfc2t", name="wfc2t")
            wfc2 = [wfc2_t[:, t, :] for t in range(FT)]
            nc.sync.dma_start(
                wfc2_t[:], d["d_wfc2"][:].rearrange("(t p) c -> p t c", p=128)
            )
            xn2T = [mlpp.tile([128, NQ], BF, tag=f"xn2T{t}", name=f"xn2T{t}") for t in range(CT)]
            hT = [mlpp.tile([128, NQ], BF, tag=f"hT{t}", name=f"hT{t}") for t in range(FT)]
            mv2 = mlpp.tile([128, 4, 2], F32, tag="mv2", name="mv2")
            rstd2 = mlpp.tile([128, 4], F32, tag="rstd2", name="rstd2")

            with (
                pool("ph4p", bufs=2, space="PSUM") as ph4p,
                pool("ph4t", bufs=2, space="PSUM") as ph4t,
                pool("ph4s", bufs=4) as ph4s,
            ):
                for qt in range(4):
                    pp = ph4p.tile([128, C], F32, tag="pp", name="pp")
                    for lo, hi in ((0, 512), (512, 768)):
                        for kt in range(KT):
                            nc.tensor.matmul(
                                pp[:, lo:hi],
                                yT[kt][:, :, qt * 128 : (qt + 1) * 128],
                                wp8[kt][:, :, lo:hi],
                                start=(kt == 0), stop=(kt == KT - 1),
                                perf_mode=DR,
                            )
                    nc.vector.tensor_tensor(
                        out=x2[:, qt, :], in0=pp[:], in1=xo32[:, qt, :], op=ALU.add
                    )
                    stats = ph4s.tile([128, 2, 6], F32, tag="bns2", name="bns2")
                    xg = x2[:, qt, :].rearrange("p (a b) -> p a b", b=384)
                    for a in range(2):
                        nc.vector.bn_stats(out=stats[:, a, :], in_=xg[:, a, :])
                    nc.vector.bn_aggr(out=mv2[:, qt, :], in_=stats[:])
                    sd2 = ph4s.tile([128, 1], F32, tag="sd2", name="sd2")
                    nc.scalar.activation(
                        out=sd2[:], in_=mv2[:, qt, 1:2], func=AF.Sqrt, bias=eps[:]
                    )
                    nc.vector.reciprocal(out=rstd2[:, qt : qt + 1], in_=sd2[:])
                for qt in range(4):
                    xn2 = ph4s.tile([128, C], BF, tag="xn2", name="xn2")
                    nc.vector.tensor_scalar(
                        out=xn2[:], in0=x2[:, qt, :], scalar1=mv2[:, qt, 0:1],
                        scalar2=rstd2[:, qt : qt + 1],
                        op0=ALU.subtract, op1=ALU.mult,
                    )
                    nc.vector.tensor_tensor(
                        out=x2[:, qt, :], in0=x2[:, qt, :], in1=bfc2b[:],
                        op=ALU.add,
                    )
                    ptb2 = ph4t.tile([128, C], F32, tag="ptb2", name="ptb2")
                    for ct in range(CT):
                        nc.tensor.matmul(
                            ptb2[:, ct * 128 : (ct + 1) * 128],
                            xn2[:, ct * 128 : (ct + 1) * 128], ident,
                            start=(ct in (0, 4)), stop=(ct in (3, 5)),
                            skip_group_check=True,
                        )
                    for ct in range(CT):
                        nc.scalar.copy(
                            out=xn2T[ct][:, qt * 128 : (qt + 1) * 128],
                            in_=ptb2[:, ct * 128 : (ct + 1) * 128],
                        )

            # =============== phase 5: fc + gelu (fp8 out) =================
            with pool("ph5p", bufs=3, space="PSUM") as ph5p:
                for f in range(FT):
                    ph_ = ph5p.tile([128, NQ], F32, tag="ph5", name="ph5")
                    for ct in range(CT):
                        nc.tensor.matmul(
                            ph_[:],
                            wfc[ct][:, f * 128 : (f + 1) * 128],
                            xn2T[ct][:],
                            start=(ct == 0), stop=(ct == CT - 1),
                        )
                    nc.scalar.activation(
                        out=hT[f][:], in_=ph_[:],
                        func=AF.Gelu_apprx_tanh,
                        bias=bfcc[:, f : f + 1],
                    )

            # =============== phase 7: fc2 + residual + out ================
            with (
                pool("ph7p", bufs=2, space="PSUM") as ph7p,
                pool("ph7s", bufs=2) as ph7s,
            ):
                for qt in range(4):
                    po = ph7p.tile([128, C], F32, tag="po", name="po")
                    for lo, hi in ((0, 512), (512, 768)):
                        for kt in range(FT):
                            nc.tensor.matmul(
                                po[:, lo:hi],
                                hT[kt][:, qt * 128 : (qt + 1) * 128],
                                wfc2[kt][:, lo:hi],
                                start=(kt == 0), stop=(kt == FT - 1),
                            )
                    ot = ph7s.tile([128, C], F32, tag="ot", name="ot")
                    nc.vector.tensor_tensor(
                        out=ot[:], in0=po[:], in1=x2[:, qt, :], op=ALU.add
                    )
                    nc.sync.dma_start(
                        d["d_out"][qt * 128 : (qt + 1) * 128, :], ot[:]
                    )


# ---------------------------------------------------------------------------
# Host-side wrapper
# ---------------------------------------------------------------------------
_PROGRAM = None


def _get_program():
    global _PROGRAM
    if _PROGRAM is None:
        _PROGRAM = build_program()
    return _PROGRAM


def make_in_maps(x, ln1_g, ln1_b, W_attn, b_attn, W_proj, b_proj,
                 ln2_g, ln2_b, W_fc, b_fc, W_fc2, b_fc2):
    x = np.asarray(x, np.float32)
    f = np.float32
    ln1_g, ln1_b = np.asarray(ln1_g, f), np.asarray(ln1_b, f)
    ln2_g, ln2_b = np.asarray(ln2_g, f), np.asarray(ln2_b, f)
    W_attn, b_attn = np.asarray(W_attn, f), np.asarray(b_attn, f)
    W_proj, b_proj = np.asarray(W_proj, f), np.asarray(b_proj, f)
    W_fc, b_fc = np.asarray(W_fc, f), np.asarray(b_fc, f)
    W_fc2, b_fc2 = np.asarray(W_fc2, f), np.asarray(b_fc2, f)

    # LN1 gain folded into W_attn rows; LN1 bias folded into the qkv biases.
    Wg = ln1_g[:, None] * W_attn
    bqkv = ln1_b @ W_attn + b_attn
    bq_eff = bqkv[0:C]
    bv_eff = bqkv[2 * C : 3 * C]
    # K bias dropped (softmax-invariant); V bias folded into b_proj.
    bp_eff = b_proj + bv_eff @ W_proj
    # LN2 gain/bias folded into W_fc / b_fc.
    Wfcg = ln2_g[:, None] * W_fc
    bfc_eff = ln2_b @ W_fc + b_fc

    ident = np.eye(128, dtype=np.float32)
    idents = np.concatenate([ident, -MBIG * ident], axis=1).astype(BF16)
    cn = np.concatenate([
        (WS * bq_eff).reshape(CT, 128).T,
        bfc_eff.reshape(FT, 128).T,
        np.broadcast_to(WS * b_fc2, (128, C)),
    ], axis=1).astype(np.float32).copy()

    shared = {
        "wq": pack_dr(Wg[:, 0:C]),
        "wk": pack_dr(Wg[:, C : 2 * C]),
        "wv": pack_dr(Wg[:, 2 * C : 3 * C]),
        "wp": pack_dr(W_proj),
        "wfc": np.asarray(Wfcg, BF16),
        "wfc2": np.asarray(WS * W_fc2, BF16),
        "idents": idents,
        "consts": cn,
    }
    in_maps, layouts = [], []
    for core in range(8):
        b, g = core // 4, core % 4
        qtiles, perm = core_layout(g)
        idx = np.concatenate([np.arange(t * 128, (t + 1) * 128) for t in perm])
        own = np.concatenate([np.arange(t * 128, (t + 1) * 128) for t in qtiles])
        m = dict(shared)
        m["x_perm"] = np.ascontiguousarray(x[b][idx]).astype(BF16)
        m["x_own32"] = (WS * (x[b][own] + bp_eff)).astype(BF16)
        m["maskrep"] = core_maskneg(qtiles, perm)
        in_maps.append(m)
        layouts.append((b, own))
    return in_maps, layouts


def unshard(results, layouts):
    out = np.empty((B, T, C), np.float32)
    for r, (b, own) in zip(results, layouts):
        out[b][own] = r["out"] * (1.0 / WS)
    return out


def kernel(**inputs):
    from concourse.bass_utils import run_bass_kernel_spmd

    nc = _get_program()
    in_maps, layouts = make_in_maps(**inputs)
    res = run_bass_kernel_spmd(nc, in_maps, core_ids=list(range(8)))
    return unshard(res.results, layouts)


# revision 43
# speedup vs baseline: 1.0384x; 1.0056x over previous
"""GPT-2 block (B=2, T=2048, C=768, H=12) on 8 Trainium2 NeuronCores.

Sharding: data-parallel over batch (2) x 4-way query-tile split per batch.
Each core computes K/V for its full batch (no on-chip collectives) and runs
attention + MLP for 4 of the 16 query tiles, interleaved {g, 7-g, 8+g, 15-g}
so the causal-attention work is identical across cores.  The SPMD program is
uniform: per-core differences are pushed into the data via a k-tile
permutation of the sequence plus per-core causal masks.

Precision/layout highlights (v2):
- QKV, proj and the attention A*V run as fp8 e4m3 DoubleRow GEMMs
  (2 k-rows/cycle); fc/fc2 stay bf16 for accuracy headroom.  Weights are
  pre-scaled by 32 on the host; LayerNorm gains/biases are folded into the
  adjacent weights/biases host-side, the K-bias is dropped
  (softmax-invariant) and the V-bias rides b_proj.
- Attention: scores are computed transposed (S^T [k,q]) in kp-PAIRS so the
  exp() output is directly the fp8 DoubleRow moving operand of the fused
  A*V matmul whose stationary is [V | 64 ones-columns] - the softmax
  denominator lands in psum rows 64:128, giving one full-partition
  reciprocal + one multiply per head pair.
- Causal masks multiply the exp output on the DVE (per-core mask data,
  uniform SPMD program); only the first in-suffix 128-query block of each
  key tile is ever non-trivial.
- The attention residual carries a 32x scale (x_own pre-scaled on host);
  LayerNorm is scale-invariant, so the scale rides through to the output,
  which the host divides by 32.
"""

import sys

sys.path.insert(0, "/opt/trn_rl_repo")

import numpy as np
import ml_dtypes

import bass_rust
import concourse.bass as bass
import concourse.bacc as bacc
import concourse.tile as tile
from concourse import mybir
from concourse.vector_clock import ScopedClock

BF16 = ml_dtypes.bfloat16
F32 = mybir.dt.float32
BF = mybir.dt.bfloat16
F8 = mybir.dt.float8e4
NP_F8 = mybir.dt.np(F8)

B, T, C, H = 2, 2048, 768, 12
HD = C // H  # 64
DFF = 4 * C  # 3072
TT = T // 128  # 16 token tiles
CT = C // 128  # 6 feature tiles
KT = C // 256  # 3 DoubleRow k-tiles over C
KT2 = DFF // 256  # 12 DoubleRow k-tiles over DFF
FT = DFF // 128  # 24
KPP = TT // 2  # 8 key-tile pairs
QPOS = (3, 7, 11, 15)  # fixed positions of this core's query tiles
NQ = 512  # queries per core
WS = 32.0  # fp8 weight pre-scale
MBIG = float(2**20)  # mask magnitude (scaled to -128 at exp input)
AF = mybir.ActivationFunctionType
ALU = mybir.AluOpType
DR = mybir.MatmulPerfMode.DoubleRow

# ---------------------------------------------------------------------------
# Tile exit-drain fix: the final SP drain carries one wait per live logical
# processor, but TRN2 ISA instructions hold at most 1 embedded sync wait in
# this toolchain. Split the waits across a chain of SP drains.
# ---------------------------------------------------------------------------
_MAX_WAITS = 1


def _drain_and_barrier(self, tick_clock, wait_clock):
    drain_inst = self.nc.sync.drain()
    wait_clock.add_sem_waits(
        drain_inst.ins, ScopedClock({None: tick_clock.global_clock})
    )
    si = drain_inst.ins.sync_info
    if si is not None and len(si.on_wait) > _MAX_WAITS:
        waits = list(si.on_wait)
        drain_inst.ins.sync_info = bass_rust.SyncInfo(
            on_wait=waits[:_MAX_WAITS], on_update=list(si.on_update)
        )
        rest = waits[_MAX_WAITS:]
        for i in range(0, len(rest), _MAX_WAITS):
            extra = self.nc.sync.drain()
            extra.ins.sync_info = bass_rust.SyncInfo(
                on_wait=rest[i : i + _MAX_WAITS], on_update=[]
            )
    self.nc.all_engine_barrier()
    assert self.sems is not None
    popped = self.nc._tile_sem_poison_stack.pop()
    assert popped is self._sem_poison
    self.nc.clear_and_free_semaphores(list(self.sems.allocated().values()))
    self.nc.all_engine_barrier()


tile.TileContext._drain_and_barrier = _drain_and_barrier


# ---------------------------------------------------------------------------
# Per-core sharding layout (host side)
# ---------------------------------------------------------------------------
def core_layout(g):
    """For group index g (0..3): (qtiles sorted, perm) with the core's query
    tiles at positions QPOS and every tile's causal prefix placed before it."""
    qtiles = sorted([g, 7 - g, 8 + g, 15 - g])
    posmap = dict(zip(QPOS, qtiles))
    rest = iter([t for t in range(TT) if t not in qtiles])
    perm = [posmap[p] if p in posmap else next(rest) for p in range(TT)]
    for j, a in enumerate(qtiles):
        assert set(range(a + 1)) <= set(perm[: QPOS[j] + 1]), (g, j, perm)
    return qtiles, perm


def core_maskneg(qtiles, perm):
    """maskrep[j, :, hi, r, :] = causal mask for k-position kp=2j+r against
    query tile bi=kp//4 (first in-suffix 128-query block), replicated over
    the head (hi) axis. Multiplied into the exp output on the DVE."""
    mrep = np.zeros((KPP, 128, 2, 2, 128), dtype=BF16)
    for kp in range(TT):
        tk = perm[kp] * 128 + np.arange(128)[:, None]
        a = qtiles[kp // 4]
        tq = a * 128 + np.arange(128)[None, :]
        m = (tk <= tq).astype(np.float32)
        mrep[kp // 2, :, 0, kp % 2, :] = m
        mrep[kp // 2, :, 1, kp % 2, :] = m
    return mrep


def pack_dr(W):
    """[K, N] fp32 -> DoubleRow-paired fp8 [K/256, 128, 2, N], pre-scaled.
    Logical k = 256*kt + 128*r + p."""
    K, N = W.shape
    Wp = (np.asarray(W, np.float32) * WS).reshape(K // 256, 2, 128, N)
    return np.ascontiguousarray(Wp.transpose(0, 2, 1, 3)).astype(NP_F8)


def _chunks(cs):
    """Query-column chunks (<=256 wide) covering [cs, NQ)."""
    if NQ - cs > 256:
        return [(cs, cs + 256), (cs + 256, NQ)]
    return [(cs, NQ)]


# ---------------------------------------------------------------------------
# The Bass program (identical for all 8 cores)
# ---------------------------------------------------------------------------
def build_program():
    nc = bacc.Bacc("TRN2")

    d_x = nc.dram_tensor("x_perm", [T, C], BF, kind="ExternalInput")
    d_xo = nc.dram_tensor("x_own32", [NQ, C], BF, kind="ExternalInput")
    d_mneg = nc.dram_tensor("maskrep", [KPP, 128, 2, 2, 128], BF, kind="ExternalInput")
    d_wq = nc.dram_tensor("wq", [KT, 128, 2, C], F8, kind="ExternalInput")
    d_wk = nc.dram_tensor("wk", [KT, 128, 2, C], F8, kind="ExternalInput")
    d_wv = nc.dram_tensor("wv", [KT, 128, 2, C], F8, kind="ExternalInput")
    d_wp = nc.dram_tensor("wp", [KT, 128, 2, C], F8, kind="ExternalInput")
    d_wfc = nc.dram_tensor("wfc", [C, DFF], BF, kind="ExternalInput")
    d_wfc2 = nc.dram_tensor("wfc2", [DFF, C], BF, kind="ExternalInput")
    # [ident | -2^20*ident]
    d_idents = nc.dram_tensor("idents", [128, 256], BF, kind="ExternalInput")
    # [bq32 (CT) | bfcc (FT) | bfc2b*32 (C)]
    d_cn = nc.dram_tensor("consts", [128, CT + FT + C], F32, kind="ExternalInput")
    d_out = nc.dram_tensor("out", [NQ, C], F32, kind="ExternalOutput")

    with tile.TileContext(nc) as tc:
        _body(nc, tc, locals())
    nc.compile()
    return nc


def _body(nc, tc, d):
    def pool(name, **kw):
        return tc.tile_pool(name=name, **kw)

    with (
        pool("const", bufs=1) as constp,
        pool("pers", bufs=1) as pers,
        pool("small", bufs=6) as small,
    ):
        # ---- constants ---------------------------------------------------
        idents = constp.tile([128, 2, 128], BF)
        nc.scalar.dma_start(
            idents[:], d["d_idents"][:].rearrange("p (a b) -> p a b", b=128)
        )
        ident, identn = idents[:, 0, :], idents[:, 1, :]
        eps = constp.tile([128, 1], F32)
        nc.vector.memset(eps[:], 1e-5)
        cn = constp.tile([128, CT + FT + C], F32)
        nc.scalar.dma_start(cn[:], d["d_cn"][:])
        bq32 = cn[:, 0:CT]
        bfcc = cn[:, CT : CT + FT]
        bfc2b = cn[:, CT + FT :]
        mrep = constp.tile([128, KPP, 2, 2, 128], BF)
        nc.gpsimd.dma_start(mrep[:], d["d_mneg"][:].rearrange("j p a r q -> p j a r q"))

        # ---- persistent tiles --------------------------------------------
        wp8 = [pers.tile([128, 2, C], F8, tag=f"wp{t}", name=f"wp{t}") for t in range(KT)]
        wfc_t = pers.tile([128, CT, DFF], BF, tag="wfct", name="wfct")
        wfc = [wfc_t[:, t, :] for t in range(CT)]
        xo32 = pers.tile([128, 4, C], BF, tag="xo", name="xo")
        kT = [pers.tile([128, T], BF, tag=f"kT{t}", name=f"kT{t}") for t in range(CT)]
        qT = [pers.tile([128, NQ], BF, tag=f"qT{t}", name=f"qT{t}") for t in range(CT)]
        xnT8 = [pers.tile([128, 2, T], F8, tag=f"xnT{t}", name=f"xnT{t}") for t in range(KT)]
        xnTq8 = [pers.tile([128, 2, NQ], F8, tag=f"xnTq{t}", name=f"xnTq{t}") for t in range(KT)]
        yT = [pers.tile([128, 2, NQ], F8, tag=f"yT{t}", name=f"yT{t}") for t in range(KT)]
        mvall = pers.tile([128, TT, 2], F32, tag="mvall", name="mvall")
        x2 = pers.tile([128, 4, C], F32, tag="x2", name="x2")
        rstd1 = pers.tile([128, TT], F32, tag="rstd1", name="rstd1")

        # ---- input DMAs (issued up-front; per-queue FIFO sets priority) --
        for t in range(KT):
            nc.scalar.dma_start(wp8[t][:], d["d_wp"][t])
        nc.gpsimd.dma_start(
            xo32[:], d["d_xo"][:].rearrange("(t p) c -> p t c", p=128)
        )

        # =============== phase 1: LN1 + transpose, feature-major fp8 ======
        wq8 = [pers.tile([128, 2, C], F8, tag=f"wq{t}", name=f"wq{t}") for t in range(KT)]
        PH1_MARKER = 1
        wk8 = [pers.tile([128, 2, C], F8, tag=f"wk{t}", name=f"wk{t}") for t in range(KT)]
        wv8 = [pers.tile([128, 2, C], F8, tag=f"wv{t}", name=f"wv{t}") for t in range(KT)]
        for t in range(KT):
            nc.scalar.dma_start(wk8[t][:], d["d_wk"][t])
        for t in range(KT):
            nc.scalar.dma_start(wv8[t][:], d["d_wv"][t])
        for t in range(KT):
            nc.scalar.dma_start(wq8[t][:], d["d_wq"][t])
        attn2_cm = tc.tile_pool(name="attn2", bufs=1)
        attn2 = attn2_cm.__enter__()
        # V2[j]: DoubleRow stationary over key pairs (tiles 2j, 2j+1):
        # per head 128 cols = [64 V | 64 ones*WS]
        V2 = [
            attn2.tile([128, 2, H * 128], F8, tag=f"V2{j}", name=f"V2{j}")
            for j in range(KPP)
        ]
        for j in range(KPP):
            v4 = V2[j][:].rearrange("p r (h e) -> p r h e", e=128)
            nc.gpsimd.memset(v4[:, :, :, HD:], WS)
        with (
            pool("ph1x", bufs=3) as ph1x,
            pool("ph1s", bufs=2) as ph1s,
        ):
            nc.gpsimd.dma_start(
                wfc_t[:], d["d_wfc"][:].rearrange("(t p) c -> p t c", p=128)
            )


            with (
                pool("ph1t", bufs=1, space="PSUM") as ph1t,
                pool("ph2k", bufs=2, space="PSUM") as ph2k,
            ):
                # PE warm-up: keep the HAM activity monitor at full clock
                warm = ph2k.tile([128, 512], F32, tag="pqk", name="warm")
                for _ in range(128):
                    nc.tensor.matmul(warm[:, 0:128], ident, ident)

                for ttg in range(4):
                    xt = ph1x.tile([128, 4, C], BF, tag="xt", name="xt")
                    nc.sync.dma_start(
                        xt[:],
                        d["d_x"][ttg * 512 : (ttg + 1) * 512, :].rearrange(
                            "(t p) c -> p t c", p=128
                        ),
                    )
                    # LN stats for the 4 tiles of this group
                    for ti in range(4):
                        tt = ttg * 4 + ti
                        stats = ph1s.tile([128, 2, 6], F32, tag="bnst", name="bnst")
                        xg = xt[:, ti, :].rearrange("p (a b) -> p a b", b=384)
                        for a in range(2):
                            nc.vector.bn_stats(out=stats[:, a, :], in_=xg[:, a, :])
                        nc.vector.bn_aggr(out=mvall[:, tt, :], in_=stats[:])
                    sd = ph1s.tile([128, 4], F32, tag="sd", name="sd")
                    nc.scalar.activation(
                        out=sd[:],
                        in_=mvall[:, 4 * ttg : 4 * ttg + 4, 1],
                        func=AF.Sqrt,
                        bias=eps[:],
                    )
                    nc.vector.reciprocal(
                        out=rstd1[:, 4 * ttg : 4 * ttg + 4], in_=sd[:]
                    )
                    xns = []
                    for ti in range(4):
                        tt = ttg * 4 + ti
                        xn = ph1s.tile([128, C], BF, tag=f"xn{ti}", name=f"xn{ti}")
                        nc.vector.tensor_scalar(
                            out=xn[:], in0=xt[:, ti, :],
                            scalar1=mvall[:, tt, 0:1],
                            scalar2=rstd1[:, tt : tt + 1],
                            op0=ALU.subtract, op1=ALU.mult,
                        )
                        xns.append(xn)
                    for half in range(2):
                        ptb = [
                            ph1t.tile([128, 512], F32, tag=f"ptb{t}", name=f"ptb{t}")
                            for t in range(3)
                        ]
                        for ti in range(4):
                            for ci, ct in enumerate(range(3 * half, 3 * half + 3)):
                                nc.tensor.matmul(
                                    ptb[ci][:, ti * 128 : (ti + 1) * 128],
                                    xns[ti][:, ct * 128 : (ct + 1) * 128],
                                    ident,
                                    start=(ti == 0), stop=(ti == 3),
                                    skip_group_check=True,
                                )
                        for ci, ct in enumerate(range(3 * half, 3 * half + 3)):
                            kt, r = ct // 2, ct % 2
                            nc.scalar.copy(
                                out=xnT8[kt][:, r, ttg * 512 : (ttg + 1) * 512],
                                in_=ptb[ci][:],
                            )
                            nc.vector.tensor_copy(
                                xnTq8[kt][:, r, ttg * 128 : (ttg + 1) * 128],
                                ptb[ci][:, 384:512],
                            )
                    # kT chunk for this token group (all 6 feature blocks)
                    for f in range(CT):
                        ps = ph2k.tile([128, 512], F32, tag="pqk", name="pk")
                        for kt in range(KT):
                            nc.tensor.matmul(
                                ps[:], wk8[kt][:, :, f * 128 : (f + 1) * 128],
                                xnT8[kt][:, :, ttg * 512 : (ttg + 1) * 512],
                                start=(kt == 0), stop=(kt == KT - 1),
                                perf_mode=DR,
                            )
                        nc.scalar.copy(
                            out=kT[f][:, ttg * 512 : (ttg + 1) * 512], in_=ps[:]
                        )
        # =============== phase 2 + 3: Q^T, V2, attention ==================
        with (
            pool("attn_sb", bufs=1) as attnp,
            pool("p4p", bufs=4) as p4p,
            pool("rrp", bufs=2) as rrp,
        ):

            with pool("ph2q", bufs=2, space="PSUM") as ph2q:
                # qT[f] = (Wq[:, f].T @ xnTq) + bq32
                for f in range(CT):
                    ps = ph2q.tile([128, NQ], F32, tag="pq", name="pq")
                    for kt in range(KT):
                        nc.tensor.matmul(
                            ps[:], wq8[kt][:, :, f * 128 : (f + 1) * 128],
                            xnTq8[kt][:], start=(kt == 0), stop=(kt == KT - 1),
                            perf_mode=DR,
                        )
                    nc.vector.tensor_scalar(
                        out=qT[f][:], in0=ps[:], scalar1=bq32[:, f : f + 1],
                        scalar2=None, op0=ALU.add,
                    )

            # ---- attention: 6 head pairs x 8 key pairs ----
            # PSUM: ps 2x2 banks + pavden 2 banks + pv 2 banks = 8
            with (
                pool("ph3s", bufs=2, space="PSUM") as ph3s,
                pool("ph3a", bufs=1, space="PSUM") as ph3a,
                pool("ph2v", bufs=1, space="PSUM") as ph2v,
            ):
                def build_v2(j):
                    for r in range(2):
                        tt = 2 * j + r
                        pv = ph2v.tile([128, C], F32, tag="pv", name="pv")
                        for lo, hi in ((0, 512), (512, 768)):
                            for kt in range(KT):
                                nc.tensor.matmul(
                                    pv[:, lo:hi],
                                    xnT8[kt][:, :, tt * 128 : (tt + 1) * 128],
                                    wv8[kt][:, :, lo:hi],
                                    start=(kt == 0), stop=(kt == KT - 1),
                                    perf_mode=DR,
                                )
                        dst = V2[j][:, r, :].rearrange("p (h e) -> p h e", e=128)[
                            :, :, 0:HD
                        ]
                        srcv = pv[:].rearrange("p (h e) -> p h e", e=HD)
                        nc.vector.tensor_copy(dst, srcv)

                for pi in range(6):  # head pair index
                    pavden = [
                        ph3a.tile([128, NQ], F32, tag=f"pav{i}", name=f"pav{i}")
                        for i in range(2)
                    ]
                    items = []
                    for j in range(KPP):
                        cs = 128 * (j // 2)
                        for ci, (c0, c1) in enumerate(_chunks(cs)):
                            items.append((j, ci, c0, c1))

                    def emit_score(idx):
                        j, ci, c0, c1 = items[idx]
                        if pi == 0 and ci == 0:
                            build_v2(j)  # just-in-time, overlaps exp
                        cs = 128 * (j // 2)
                        w = c1 - c0
                        psb = ph3s.tile([128, 2, 2, 256], F32, tag="ps", name="ps")
                        ps = psb[:, :, :, 0:w] if w < 256 else psb[:]
                        has_mask = c0 == cs
                        for hi in range(2):
                            for r in range(2):
                                kp = 2 * j + r
                                nc.tensor.matmul(
                                    ps[:, hi, r, :],
                                    kT[pi][
                                        hi * 64 : hi * 64 + 64,
                                        kp * 128 : (kp + 1) * 128,
                                    ],
                                    qT[pi][hi * 64 : hi * 64 + 64, c0:c1],
                                    start=(r == 0),
                                    stop=(hi == 1 and r == 1),
                                    skip_group_check=True,
                                )
                        p4b = p4p.tile([128, 2, 2, 256], F8, tag="p4", name="p4")
                        p4 = p4b[:, :, :, 0:w] if w < 256 else p4b[:]
                        nc.scalar.activation(
                            out=p4[:], in_=ps[:], func=AF.Exp,
                            scale=0.125 / (WS * WS),
                        )
                        if has_mask:
                            nc.vector.tensor_tensor(
                                out=p4[:, :, :, 0:128], in0=p4[:, :, :, 0:128],
                                in1=mrep[:, j, :, :, :], op=ALU.mult,
                            )
                        return p4

                    def emit_av(idx, p4):
                        j, ci, c0, c1 = items[idx]
                        for hi in range(2):
                            h = 2 * pi + hi
                            nc.tensor.matmul(
                                pavden[hi][:, c0:c1],
                                V2[j][:, :, h * 128 : (h + 1) * 128],
                                p4[:, hi, :, :],
                                start=(idx == 0),
                                stop=(idx == len(items) - 1),
                                perf_mode=DR,
                                skip_group_check=True,
                            )

                    prev = None
                    for idx in range(len(items)):
                        p4 = emit_score(idx)
                        if prev is not None:
                            emit_av(idx - 1, prev)
                        prev = p4
                    emit_av(len(items) - 1, prev)
                    # normalize: rows 64:128 of pavden hold den * 64
                    rr = rrp.tile([128, NQ], F32, tag="rr", name="rr")
                    rr2 = rrp.tile([128, NQ], F32, tag="rr2", name="rr2")
                    nc.vector.tensor_copy(rr2[0:64, :], pavden[0][64:128, :])
                    nc.vector.tensor_copy(rr2[64:128, :], pavden[1][64:128, :])
                    nc.vector.reciprocal_approx_fast(out=rr[:], in_=rr2[:])
                    kt, r = pi // 2, pi % 2
                    nc.vector.tensor_tensor(
                        out=yT[kt][0:64, r, :], in0=pavden[0][0:64, :],
                        in1=rr[0:64, :], op=ALU.mult,
                    )
                    nc.vector.tensor_tensor(
                        out=yT[kt][64:128, r, :], in0=pavden[1][0:64, :],
                        in1=rr[64:128, :], op=ALU.mult,
                    )

        attn2_cm.__exit__(None, None, None)

        # =============== phase 4: proj + residual + LN2 ===============
        with pool("mlp_sb", bufs=1) as mlpp:
            wfc2_t = mlpp.tile([128, FT, C], BF, tag="wfc2t", name="wfc2t")
            wfc2 = [wfc2_t[:, t, :] for t in range(FT)]
            nc.sync.dma_start(
                wfc2_t[:], d["d_wfc2"][:].rearrange("(t p) c -> p t c", p=128)
            )
            xn2T = [mlpp.tile([128, NQ], BF, tag=f"xn2T{t}", name=f"xn2T{t}") for t in range(CT)]
            hT = [mlpp.tile([128, NQ], BF, tag=f"hT{t}", name=f"hT{t}") for t in range(FT)]
            mv2 = mlpp.tile([128, 4, 2], F32, tag="mv2", name="mv2")
            rstd2 = mlpp.tile([128, 4], F32, tag="rstd2", name="rstd2")

            with (
                pool("ph4p", bufs=2, space="PSUM") as ph4p,
                pool("ph4t", bufs=2, space="PSUM") as ph4t,
                pool("ph4s", bufs=4) as ph4s,
            ):
                for qt in range(4):
                    pp = ph4p.tile([128, C], F32, tag="pp", name="pp")
                    for lo, hi in ((0, 512), (512, 768)):
                        for kt in range(KT):
                            nc.tensor.matmul(
                                pp[:, lo:hi],
                                yT[kt][:, :, qt * 128 : (qt + 1) * 128],
                                wp8[kt][:, :, lo:hi],
                                start=(kt == 0), stop=(kt == KT - 1),
                                perf_mode=DR,
                            )
                    nc.vector.tensor_tensor(
                        out=x2[:, qt, :], in0=pp[:], in1=xo32[:, qt, :], op=ALU.add
                    )
                    stats = ph4s.tile([128, 2, 6], F32, tag="bns2", name="bns2")
                    xg = x2[:, qt, :].rearrange("p (a b) -> p a b", b=384)
                    for a in range(2):
                        nc.vector.bn_stats(out=stats[:, a, :], in_=xg[:, a, :])
                    nc.vector.bn_aggr(out=mv2[:, qt, :], in_=stats[:])
                    sd2 = ph4s.tile([128, 1], F32, tag="sd2", name="sd2")
                    nc.scalar.activation(
                        out=sd2[:], in_=mv2[:, qt, 1:2], func=AF.Sqrt, bias=eps[:]
                    )
                    nc.vector.reciprocal(out=rstd2[:, qt : qt + 1], in_=sd2[:])
                for qt in range(4):
                    xn2 = ph4s.tile([128, C], BF, tag="xn2", name="xn2")
                    nc.vector.tensor_scalar(
                        out=xn2[:], in0=x2[:, qt, :], scalar1=mv2[:, qt, 0:1],
                        scalar2=rstd2[:, qt : qt + 1],
                        op0=ALU.subtract, op1=ALU.mult,
                    )
                    nc.vector.tensor_tensor(
                        out=x2[:, qt, :], in0=x2[:, qt, :], in1=bfc2b[:],
                        op=ALU.add,
                    )
                    ptb2 = ph4t.tile([128, C], F32, tag="ptb2", name="ptb2")
                    for ct in range(CT):
                        nc.tensor.matmul(
                            ptb2[:, ct * 128 : (ct + 1) * 128],
                            xn2[:, ct * 128 : (ct + 1) * 128], ident,
                            start=(ct in (0, 4)), stop=(ct in (3, 5)),
                            skip_group_check=True,
                        )
                    for ct in range(CT):
                        nc.scalar.copy(
                            out=xn2T[ct][:, qt * 128 : (qt + 1) * 128],
                            in_=ptb2[:, ct * 128 : (ct + 1) * 128],
                        )

            # =============== phase 5: fc + gelu (fp8 out) =================
            with pool("ph5p", bufs=3, space="PSUM") as ph5p:
                for f in range(FT):
                    ph_ = ph5p.tile([128, NQ], F32, tag="ph5", name="ph5")
                    for ct in range(CT):
                        nc.tensor.matmul(
                            ph_[:],
                            wfc[ct][:, f * 128 : (f + 1) * 128],
                            xn2T[ct][:],
                            start=(ct == 0), stop=(ct == CT - 1),
                        )
                    nc.scalar.activation(
                        out=hT[f][:], in_=ph_[:],
                        func=AF.Gelu_apprx_tanh,
                        bias=bfcc[:, f : f + 1],
                    )

            # =============== phase 7: fc2 + residual + out ================
            with (
                pool("ph7p", bufs=2, space="PSUM") as ph7p,
                pool("ph7s", bufs=2) as ph7s,
            ):
                for qt in range(4):
                    po = ph7p.tile([128, C], F32, tag="po", name="po")
                    for lo, hi in ((0, 512), (512, 768)):
                        for kt in range(FT):
                            nc.tensor.matmul(
                                po[:, lo:hi],
                                hT[kt][:, qt * 128 : (qt + 1) * 128],
                                wfc2[kt][:, lo:hi],
                                start=(kt == 0), stop=(kt == FT - 1),
                            )
                    ot = ph7s.tile([128, C], F32, tag="ot", name="ot")
                    nc.vector.tensor_tensor(
                        out=ot[:], in0=po[:], in1=x2[:, qt, :], op=ALU.add
                    )
                    nc.sync.dma_start(
                        d["d_out"][qt * 128 : (qt + 1) * 128, :], ot[:]
                    )


# ---------------------------------------------------------------------------
# Host-side wrapper
# ---------------------------------------------------------------------------
_PROGRAM = None


def _get_program():
    global _PROGRAM
    if _PROGRAM is None:
        _PROGRAM = build_program()
    return _PROGRAM


def make_in_maps(x, ln1_g, ln1_b, W_attn, b_attn, W_proj, b_proj,
                 ln2_g, ln2_b, W_fc, b_fc, W_fc2, b_fc2):
    x = np.asarray(x, np.float32)
    f = np.float32
    ln1_g, ln1_b = np.asarray(ln1_g, f), np.asarray(ln1_b, f)
    ln2_g, ln2_b = np.asarray(ln2_g, f), np.asarray(ln2_b, f)
    W_attn, b_attn = np.asarray(W_attn, f), np.asarray(b_attn, f)
    W_proj, b_proj = np.asarray(W_proj, f), np.asarray(b_proj, f)
    W_fc, b_fc = np.asarray(W_fc, f), np.asarray(b_fc, f)
    W_fc2, b_fc2 = np.asarray(W_fc2, f), np.asarray(b_fc2, f)

    # LN1 gain folded into W_attn rows; LN1 bias folded into the qkv biases.
    Wg = ln1_g[:, None] * W_attn
    bqkv = ln1_b @ W_attn + b_attn
    bq_eff = bqkv[0:C]
    bv_eff = bqkv[2 * C : 3 * C]
    # K bias dropped (softmax-invariant); V bias folded into b_proj.
    bp_eff = b_proj + bv_eff @ W_proj
    # LN2 gain/bias folded into W_fc / b_fc.
    Wfcg = ln2_g[:, None] * W_fc
    bfc_eff = ln2_b @ W_fc + b_fc

    ident = np.eye(128, dtype=np.float32)
    idents = np.concatenate([ident, -MBIG * ident], axis=1).astype(BF16)
    cn = np.concatenate([
        (WS * bq_eff).reshape(CT, 128).T,
        bfc_eff.reshape(FT, 128).T,
        np.broadcast_to(WS * b_fc2, (128, C)),
    ], axis=1).astype(np.float32).copy()

    shared = {
        "wq": pack_dr(Wg[:, 0:C]),
        "wk": pack_dr(Wg[:, C : 2 * C]),
        "wv": pack_dr(Wg[:, 2 * C : 3 * C]),
        "wp": pack_dr(W_proj),
        "wfc": np.asarray(Wfcg, BF16),
        "wfc2": np.asarray(WS * W_fc2, BF16),
        "idents": idents,
        "consts": cn,
    }
    in_maps, layouts = [], []
    for core in range(8):
        b, g = core // 4, core % 4
        qtiles, perm = core_layout(g)
        idx = np.concatenate([np.arange(t * 128, (t + 1) * 128) for t in perm])
        own = np.concatenate([np.arange(t * 128, (t + 1) * 128) for t in qtiles])
        m = dict(shared)
        m["x_perm"] = np.ascontiguousarray(x[b][idx]).astype(BF16)
        m["x_own32"] = (WS * (x[b][own] + bp_eff)).astype(BF16)
        m["maskrep"] = core_maskneg(qtiles, perm)
        in_maps.append(m)
        layouts.append((b, own))
    return in_maps, layouts


def unshard(results, layouts):
    out = np.empty((B, T, C), np.float32)
    for r, (b, own) in zip(results, layouts):
        out[b][own] = r["out"] * (1.0 / WS)
    return out


def kernel(**inputs):
    from concourse.bass_utils import run_bass_kernel_spmd

    nc = _get_program()
    in_maps, layouts = make_in_maps(**inputs)
    res = run_bass_kernel_spmd(nc, in_maps, core_ids=list(range(8)))
    return unshard(res.results, layouts)
